# revision 66
# baseline (speedup 1.0000x reference)
"""Trainium2 Bass kernel for a 4-head GAT layer (N=4096, D=256, O=64, H=4).

Math (reference):
    feat[h] = X @ W[h]                                  [N, O]
    s[h,i] = feat[h,i] @ a_src[h],  t[h,j] = feat[h,j] @ a_dst[h]
    score[h,i,j] = leaky_relu(s_i + t_j, 0.2), masked by A>0, softmax over j
    out[i, h*O+o] = sum_j attn[h,i,j] feat[h,j,o] + b[h,o]

Exact decomposition used on device (multiply num+den by e^{-0.2 s_i}):
    exp(leaky(x)) * e^{-0.2 s} = q_j + [x>=0] * (e^{0.8 s_i} v_j - q_j)
      with q_j = e^{0.2 t_j}, v_j = e^{t_j}
    numer'_i = (A @ qP)_i + e^{0.8 s_i} (M2 @ vP)_i - (M2 @ qP)_i
    where M2 = A * [s_i + t_j >= 0]; panels carry [feat*z | z] columns so the
    row-sum (softmax denominator) rides along as column 64 of each group.

Implementation notes (engine balance is the whole game here):
  - Everything is f16: per-j-tile panels [q_h | v_h] hold z_j*[feat_j | 1];
    the A-branch matmul reuses the q slots via a strided [p, 4, 68] rhs, so
    the q panel is built once.  PSUM accumulators are pre-zeroed and all
    matmuls run start=False: a start=True re-marks its whole PSUM tile
    pending-zero and wipes the other chains sharing the tile.
  - M2 masks: mb = step(s_i + t_j) via DVE is_le (~37%) or saturated
    sigmoid on the Scalar engine (~63%, tuned so DVE and ACT finish
    together), then one pair-wide DVE multiply by A.  NEGS[p,i] = -s_i is a
    broadcast-row tile from a matmul against a repeated -w_src panel.
  - The masks for pair q+1 are built while pair q's matmuls run; GpSimd only
    gets small off-critical-path work (it is ~2-3x slower per element and
    poisons SBUF bandwidth when loaded).
Sharding: destination rows split 512/core across 8 cores; source features
recomputed per core.  No collectives.  b added on host (always zero here).
"""

from contextlib import ExitStack

import numpy as np

import concourse.bass as bass
import concourse.tile as tile
import concourse.mybir as mybir
from concourse import bacc
from concourse.bass_utils import run_bass_kernel_spmd

P = 128
IN_DIM = 256
OUT_DIM = 64
HEADS = 4
N_TOTAL = 4096
N_CORES = 8
ROWS = N_TOTAL // N_CORES  # 512

F32 = mybir.dt.float32
F16 = mybir.dt.float16

AL = mybir.AluOpType
AF = mybir.ActivationFunctionType

GRP = 68                    # [f(64) | den | pad(3)] per panel group
VQBLK = HEADS * 2 * GRP     # 544: per-head [q|v] panel cols per source tile


def build_program(n_total=N_TOTAL, rows=ROWS, num_devices=N_CORES):
    ntiles = n_total // P       # 32 source tiles
    npairs = ntiles // 2        # 16 DoubleRow pairs
    nib = rows // P             # 4 destination blocks

    nc = bacc.Bacc("TRN2", target_bir_lowering=False, debug=False,
                   num_devices=num_devices)

    XT = nc.dram_tensor("XT", [IN_DIM, n_total], F16, kind="ExternalInput")
    XTOWN = nc.dram_tensor("XTOWN", [IN_DIM, rows], F16, kind="ExternalInput")
    W8 = nc.dram_tensor("W8", [IN_DIM, 260], F16, kind="ExternalInput")
    W4 = nc.dram_tensor("W4", [IN_DIM, 4], F16, kind="ExternalInput")
    WSRCB = nc.dram_tensor("WSRCB", [IN_DIM, 4 * P], F16, kind="ExternalInput")
    AT16 = nc.dram_tensor("AT16", [n_total, rows], F16, kind="ExternalInput")
    OUT = nc.dram_tensor("OUT", [rows, HEADS * OUT_DIM], F32,
                         kind="ExternalOutput")

    with tile.TileContext(nc) as tc, ExitStack() as ctx:
        big = ctx.enter_context(tc.tile_pool(name="big", bufs=1))

        # ---- Phase 0: DMA loads (small tensors first) ----
        xtown_sb = big.tile([P, 2 * rows], F16, tag="xtown")
        for d in range(2):
            nc.sync.dma_start(xtown_sb[:, d * rows:(d + 1) * rows],
                              XTOWN[d * P:(d + 1) * P, :])
        w8_sb = big.tile([P, 2 * 260], F16, tag="w8")
        for d in range(2):
            nc.sync.dma_start(w8_sb[:, d * 260:(d + 1) * 260],
                              W8[d * P:(d + 1) * P, :])
        w4_sb = big.tile([P, 2 * 4], F16, tag="w4")
        for d in range(2):
            nc.sync.dma_start(w4_sb[:, d * 4:(d + 1) * 4],
                              W4[d * P:(d + 1) * P, :])
        wsrcb_sb = big.tile([P, 2 * 4 * P], F16, tag="wsrcb")
        for d in range(2):
            nc.sync.dma_start(wsrcb_sb[:, d * 4 * P:(d + 1) * 4 * P],
                              WSRCB[d * P:(d + 1) * P, :])
        xt_sb = big.tile([P, 2 * n_total], F16, tag="xt")
        nch = 8
        for c in range(nch):
            w = n_total // nch
            for d in range(2):
                nc.sync.dma_start(
                    xt_sb[:, d * n_total + c * w: d * n_total + (c + 1) * w],
                    XT[d * P:(d + 1) * P, c * w:(c + 1) * w])
        at16_sb = big.tile([P, ntiles * rows], F16, tag="at16")
        for jt in range(ntiles):
            nc.sync.dma_start(at16_sb[:, jt * rows:(jt + 1) * rows],
                              AT16[jt * P:(jt + 1) * P, :])

        # ---- SBUF working tiles ----
        negs = big.tile([P, HEADS * rows], F16, tag="negs")     # -s bcast rows
        t3 = big.tile([P, ntiles * 4], F32, tag="t3")           # t per (nt,h)
        s_own = big.tile([P, nib * 4], F32, tag="sown")
        u8 = big.tile([P, nib * 4], F32, tag="u8")              # e^{0.8 s}
        qv = big.tile([P, ntiles * 4], F32, tag="qv")           # e^{0.2 t}
        vv = big.tile([P, ntiles * 4], F32, tag="vv")           # e^{t}
        tbig = big.tile([P, ntiles * 4], F32, tag="tbig")       # 1e4 * t
        fe66 = big.tile([P, ntiles * 4 * 66], F16, tag="fe66")  # feat + ones
        vq16 = big.tile([P, ntiles * VQBLK], F16, tag="vq16")   # panels [q|v]
        vq4 = vq16[:].rearrange("p (n h c) -> p n h c", h=4, c=2 * GRP)
        out_sbs = []
        for ib in range(nib):
            osb = big.tile([P, HEADS * OUT_DIM], F32, tag=f"osb{ib}")
            out_sbs.append(osb)

        # ---- Phase 1a: NEGS + s_own (small matmuls on XTOWN) ----
        with tc.tile_pool(name="pneg", bufs=2, space=bass.MemorySpace.PSUM) as pn, \
             tc.tile_pool(name="pso", bufs=1, space=bass.MemorySpace.PSUM) as pso:
            for h in range(HEADS):
                ps = pn.tile([P, rows], F32, tag="ps_neg")
                for d in range(2):
                    nc.tensor.matmul(
                        ps[:],
                        wsrcb_sb[:, d * 4 * P + h * P: d * 4 * P + (h + 1) * P],
                        xtown_sb[:, d * rows:(d + 1) * rows],
                        start=(d == 0), stop=(d == 1))
                nc.scalar.activation(negs[:, h * rows:(h + 1) * rows],
                                     ps[:], AF.Copy)
            ps_s = pso.tile([P, nib * 4], F32, tag="ps_s")
            for ib in range(nib):
                for d in range(2):
                    nc.tensor.matmul(
                        ps_s[:, ib * 4:(ib + 1) * 4],
                        xtown_sb[:, d * rows + ib * P: d * rows + (ib + 1) * P],
                        w4_sb[:, d * 4:(d + 1) * 4],
                        start=(d == 0), stop=(d == 1))
            nc.vector.tensor_copy(s_own[:], ps_s[:])
        nc.scalar.activation(u8[:], s_own[:], AF.Exp, scale=0.8)

        # panel pad pre-zero (pads 65:68 of each group feed matmuls)
        nc.vector.memset(vq4[:, :, :, 0:2 * GRP].rearrange(
            "p n h (f c) -> p n h f c", c=GRP)[:, :, :, :, 65:68], 0.0)
        # fe66 ones column (65th col = 1.0 so z-mult emits the den column)
        fe66v = fe66[:].rearrange("p (n h c) -> p n h c", h=4, c=66)
        nc.vector.memset(fe66v[:, :, :, 64:65], 1.0)

        # masks: built interleaved with phase 1b (need only negs/t3/at);
        # all 64 tiles stay resident so the matmul loop never waits long.
        m2_pool = ctx.enter_context(tc.tile_pool(name="m2", bufs=8))
        mb_pool = ctx.enter_context(tc.tile_pool(name="mb", bufs=8))
        at16_3 = at16_sb[:].rearrange("p (j i) -> p j i", i=rows)
        SIG_HEADS = ()     # heads whose masks go ACT-sigmoid -> Pool-mult
        m2_tiles = {}

        def make_masks(q):
            tiles = []
            for h in range(HEADS):
                m2t = m2_pool.tile([P, 2 * rows], F16, tag="m2t")
                mbt = mb_pool.tile([P, 2 * rows], F16, tag="mbt")
                for half in range(2):
                    jt = 2 * q + half
                    mbh = mbt[:, half * rows:(half + 1) * rows]
                    if h >= 2 or (h == 1 and half == 0):
                        # step mask via saturated sigmoid on the ACT engine
                        nc.scalar.activation(
                            mbh, negs[:, h * rows:(h + 1) * rows],
                            AF.Sigmoid, scale=-1.0e4,
                            bias=tbig[:, jt * 4 + h: jt * 4 + h + 1])
                    else:
                        nc.vector.tensor_scalar(
                            mbh, negs[:, h * rows:(h + 1) * rows],
                            t3[:, jt * 4 + h: jt * 4 + h + 1],
                            None, AL.is_le)
                # pair-wide multiply by A (one 1024-col 2x op)
                nc.vector.tensor_tensor(
                    m2t[:], mbt[:],
                    at16_sb[:, (2 * q) * rows:(2 * q + 2) * rows], AL.mult)
                tiles.append(m2t)
            m2_tiles[q] = tiles

        # ---- Phase 1b: feat matmuls -> fe66/t3; exps; fp8 panels; masks ----
        CH = 4  # nt per exp chunk
        with tc.tile_pool(name="pfeat", bufs=6, space=bass.MemorySpace.PSUM) as pf:
            for nt0 in range(0, ntiles, CH):
                for nt in range(nt0, nt0 + CH):
                    ps = pf.tile([P, 260], F32, tag="ps_f")
                    for d in range(2):
                        nc.tensor.matmul(
                            ps[:, 0:260],
                            xt_sb[:, d * n_total + nt * P: d * n_total + (nt + 1) * P],
                            w8_sb[:, d * 260:(d + 1) * 260],
                            start=(d == 0), stop=(d == 1))
                    nc.scalar.activation(t3[:, nt * 4:(nt + 1) * 4],
                                         ps[:, 256:260], AF.Copy)
                    nc.scalar.activation(
                        fe66v[:, nt, :, 0:64],
                        ps[:, 0:256].rearrange("p (h c) -> p h c", c=64),
                        AF.Copy)
                ch = slice(nt0 * 4, (nt0 + CH) * 4)
                nc.scalar.activation(qv[:, ch], t3[:, ch], AF.Exp, scale=0.2)
                nc.scalar.activation(vv[:, ch], t3[:, ch], AF.Exp)
                # panels for these source tiles: fp8 q-block (A-branch,
                # wide broadcast on DVE) + f16 [q|v] groups (M-branch)
                for nt in range(nt0, nt0 + CH):
                    for h in range(HEADS):
                        feh = fe66v[:, nt, h, 0:65]
                        nc.vector.tensor_scalar_mul(
                            vq4[:, nt, h, 0:65], feh,
                            qv[:, nt * 4 + h: nt * 4 + h + 1])
                        if (nt + h) % 2 == 0:
                            nc.scalar.activation(
                                vq4[:, nt, h, GRP:GRP + 65], feh, AF.Copy,
                                scale=vv[:, nt * 4 + h: nt * 4 + h + 1])
                        else:
                            nc.vector.tensor_scalar_mul(
                                vq4[:, nt, h, GRP:GRP + 65], feh,
                                vv[:, nt * 4 + h: nt * 4 + h + 1])

        # ---- Phase 2: masked matmuls over j-pairs ----
        nc.vector.tensor_scalar_mul(tbig[:], t3[:], 1.0e4)
        with tc.tile_pool(name="pacc", bufs=1, space=bass.MemorySpace.PSUM) as pa:
            # per ib: tile1 = [A(272) | Mh0(136)], tile2 = [Mh1|Mh2|Mh3]
            t1 = []
            t2 = []
            for ib in range(nib):
                t1_ib = pa.tile([P, 408], F32, tag=f"t1_{ib}")
                t1.append(t1_ib)
            for ib in range(nib):
                t2_ib = pa.tile([P, 408], F32, tag=f"t2_{ib}")
                t2.append(t2_ib)
            # Pre-zero accumulators; all matmuls run start=False because a
            # start=True re-marks the whole tile pending-zero and wipes the
            # other chains' first-pair contributions (3 chains share a tile).
            zsrc = big.tile([P, 408], F32, tag="zsrc")
            nc.vector.memset(zsrc[:], 0.0)
            for ib in range(nib):
                nc.vector.memset(t1[ib][:], 0.0)
                nc.scalar.activation(t2[ib][:], zsrc[:], AF.Copy)

            make_masks(0)
            for q in range(npairs):
                m2_cur = m2_tiles[q]
                if q + 1 < npairs:
                    make_masks(q + 1)
                st = False
                sp = (q == npairs - 1)
                pr = slice(2 * q, 2 * q + 2)
                for ib in range(nib):
                    for half in range(2):
                        jt = 2 * q + half
                        lhsA = at16_3[:, jt, ib * P:(ib + 1) * P]
                        nc.tensor.matmul(t1[ib][:, 0:272], lhsA,
                                         vq4[:, jt, :, 0:GRP],
                                         start=st, stop=sp,
                                         skip_group_check=True)
                    for h in range(HEADS):
                        dst = (t1[ib][:, 272:408] if h == 0
                               else t2[ib][:, (h - 1) * 136: h * 136])
                        for half in range(2):
                            jt = 2 * q + half
                            m2ap = m2_cur[h][
                                :, half * rows + ib * P:
                                half * rows + (ib + 1) * P]
                            rvq = vq4[:, jt, h, 0:136]
                            nc.tensor.matmul(dst, m2ap, rvq,
                                             start=st, stop=sp,
                                             skip_group_check=True)

            # ---- Epilogue: PSUM->SBUF on ACT, combine from SBUF on DVE/Pool ----
            with tc.tile_pool(name="epi", bufs=4) as ep:
                for ib in range(nib):
                    s1 = ep.tile([P, 408], F32, tag="s1")
                    s2 = ep.tile([P, 408], F32, tag="s2")
                    nc.scalar.activation(s1[:], t1[ib][:], AF.Copy)
                    nc.scalar.activation(s2[:], t2[ib][:], AF.Copy)
                    for h in range(HEADS):
                        pa_h = s1[:, h * GRP: h * GRP + 65]
                        mh = (s1[:, 272:408] if h == 0
                              else s2[:, (h - 1) * 136: h * 136])
                        pmq = mh[:, 0:65]
                        pmv = mh[:, GRP:GRP + 65]
                        tmp = ep.tile([P, 65], F32, tag="tmp")
                        nc.vector.scalar_tensor_tensor(
                            tmp[:], pmv, u8[:, ib * 4 + h: ib * 4 + h + 1],
                            pmq, AL.mult, AL.subtract)
                        numer = ep.tile([P, 65], F32, tag="num")
                        nc.gpsimd.tensor_tensor(numer[:], tmp[:], pa_h, AL.add)
                        rc = ep.tile([P, 1], F32, tag="rc")
                        nc.vector.reciprocal(rc[:], numer[:, 64:65])
                        nc.vector.tensor_scalar_mul(
                            out_sbs[ib][:, h * OUT_DIM:(h + 1) * OUT_DIM],
                            numer[:, 0:OUT_DIM], rc[:])
        for ib in range(nib):
            nc.sync.dma_start(OUT[ib * P:(ib + 1) * P, :], out_sbs[ib][:])

    nc.compile()
    return nc


def prep_inputs(X, A, W, a, n_total=N_TOTAL, rows=ROWS, n_cores=N_CORES):
    f16 = np.float16
    X = np.asarray(X, np.float32)
    A = np.asarray(A)
    W = np.asarray(W, np.float32)
    a = np.asarray(a, np.float32)

    XT = np.ascontiguousarray(X.T).astype(f16)
    Wcat = np.ascontiguousarray(W.transpose(1, 0, 2).reshape(IN_DIM, HEADS * OUT_DIM))
    a_src, a_dst = a[:, :OUT_DIM], a[:, OUT_DIM:]
    w_src = np.einsum('hdo,ho->hd', W, a_src).astype(np.float32)
    w_dst = np.einsum('hdo,ho->hd', W, a_dst).astype(np.float32)
    W8 = np.concatenate([Wcat, w_dst.T], axis=1).astype(f16)
    W4 = np.ascontiguousarray(w_src.T).astype(f16)
    WSRCB = np.repeat(-w_src.T[:, :, None], P, axis=2).reshape(IN_DIM, HEADS * P)
    WSRCB = np.ascontiguousarray(WSRCB).astype(f16)

    Af16 = (A > 0).astype(f16)
    in_maps = []
    for c in range(n_cores):
        i0 = c * rows
        at16 = np.ascontiguousarray(Af16[i0:i0 + rows, :].T)
        xtown = np.ascontiguousarray(X[i0:i0 + rows, :].T).astype(f16)
        in_maps.append({
            "XT": XT, "XTOWN": xtown, "W8": W8, "W4": W4,
            "WSRCB": WSRCB, "AT16": at16,
        })
    return in_maps


_CACHED_NC = None


def _get_nc():
    global _CACHED_NC
    if _CACHED_NC is None:
        _CACHED_NC = build_program()
    return _CACHED_NC


def kernel(X, A, W, a, b, _trace=False, _trace_kwargs=None):
    nc = _get_nc()
    in_maps = prep_inputs(X, A, W, a)
    kw = {}
    if _trace:
        kw["trace"] = True
        if _trace_kwargs:
            kw.update(_trace_kwargs)
    res = run_bass_kernel_spmd(nc, in_maps, core_ids=list(range(N_CORES)), **kw)
    out = np.concatenate([r["OUT"] for r in res.results], axis=0)
    out = out + np.asarray(b, np.float32).reshape(1, HEADS * OUT_DIM)
    if _trace:
        return out.astype(np.float32), res
    return out.astype(np.float32)


# revision 67
# speedup vs baseline: 1.0654x; 1.0654x over previous
"""Trainium2 Bass kernel for a 4-head GAT layer (N=4096, D=256, O=64, H=4).

Math (reference):
    feat[h] = X @ W[h]                                  [N, O]
    s[h,i] = feat[h,i] @ a_src[h],  t[h,j] = feat[h,j] @ a_dst[h]
    score[h,i,j] = leaky_relu(s_i + t_j, 0.2), masked by A>0, softmax over j
    out[i, h*O+o] = sum_j attn[h,i,j] feat[h,j,o] + b[h,o]

Exact decomposition used on device (multiply num+den by e^{-0.2 s_i}):
    exp(leaky(x)) * e^{-0.2 s} = q_j + [x>=0] * (e^{0.8 s_i} v_j - q_j)
      with q_j = e^{0.2 t_j}, v_j = e^{t_j}
    numer'_i = (A @ qP)_i + e^{0.8 s_i} (M2 @ vP)_i - (M2 @ qP)_i
    where M2 = A * [s_i + t_j >= 0]; panels carry [feat*z | z] columns so the
    row-sum (softmax denominator) rides along as column 64 of each group.

Implementation notes (engine balance is the whole game here):
  - Everything is f16: per-j-tile panels [q_h | v_h] hold z_j*[feat_j | 1];
    the A-branch matmul reuses the q slots via a strided [p, 4, 68] rhs, so
    the q panel is built once.  PSUM accumulators are pre-zeroed and all
    matmuls run start=False: a start=True re-marks its whole PSUM tile
    pending-zero and wipes the other chains sharing the tile.
  - M2 masks: mb = step(s_i + t_j) via DVE is_le (~37%) or saturated
    sigmoid on the Scalar engine (~63%, tuned so DVE and ACT finish
    together), then one pair-wide DVE multiply by A.  NEGS[p,i] = -s_i is a
    broadcast-row tile from a matmul against a repeated -w_src panel.
  - The masks for pair q+1 are built while pair q's matmuls run; GpSimd only
    gets small off-critical-path work (it is ~2-3x slower per element and
    poisons SBUF bandwidth when loaded).
Sharding: destination rows split 512/core across 8 cores; source features
recomputed per core.  No collectives.  b added on host (always zero here).
"""

from contextlib import ExitStack

import numpy as np

import concourse.bass as bass
import concourse.tile as tile
import concourse.mybir as mybir
from concourse import bacc
from concourse.bass_utils import run_bass_kernel_spmd

P = 128
IN_DIM = 256
OUT_DIM = 64
HEADS = 4
N_TOTAL = 4096
N_CORES = 8
ROWS = N_TOTAL // N_CORES  # 512

F32 = mybir.dt.float32
F16 = mybir.dt.float16

AL = mybir.AluOpType
AF = mybir.ActivationFunctionType

GRP = 68                    # [f(64) | den | pad(3)] per panel group
VQBLK = HEADS * 2 * GRP     # 544: per-head [q|v] panel cols per source tile


def build_program(n_total=N_TOTAL, rows=ROWS, num_devices=N_CORES):
    ntiles = n_total // P       # 32 source tiles
    npairs = ntiles // 2        # 16 DoubleRow pairs
    nib = rows // P             # 4 destination blocks

    nc = bacc.Bacc("TRN2", target_bir_lowering=False, debug=False,
                   num_devices=num_devices)

    XT = nc.dram_tensor("XT", [IN_DIM, n_total], F16, kind="ExternalInput")
    XTOWN = nc.dram_tensor("XTOWN", [IN_DIM, rows], F16, kind="ExternalInput")
    W8 = nc.dram_tensor("W8", [IN_DIM, 260], F16, kind="ExternalInput")
    W4 = nc.dram_tensor("W4", [IN_DIM, 4], F16, kind="ExternalInput")
    WSRCB = nc.dram_tensor("WSRCB", [IN_DIM, 4 * P], F16, kind="ExternalInput")
    AT16 = nc.dram_tensor("AT16", [n_total, rows], F16, kind="ExternalInput")
    OUT = nc.dram_tensor("OUT", [rows, HEADS * OUT_DIM], F32,
                         kind="ExternalOutput")

    with tile.TileContext(nc) as tc, ExitStack() as ctx:
        big = ctx.enter_context(tc.tile_pool(name="big", bufs=1))

        # ---- Phase 0: DMA loads (small tensors first) ----
        xtown_sb = big.tile([P, 2 * rows], F16, tag="xtown")
        for d in range(2):
            nc.sync.dma_start(xtown_sb[:, d * rows:(d + 1) * rows],
                              XTOWN[d * P:(d + 1) * P, :])
        w8_sb = big.tile([P, 2 * 260], F16, tag="w8")
        for d in range(2):
            nc.sync.dma_start(w8_sb[:, d * 260:(d + 1) * 260],
                              W8[d * P:(d + 1) * P, :])
        w4_sb = big.tile([P, 2 * 4], F16, tag="w4")
        for d in range(2):
            nc.sync.dma_start(w4_sb[:, d * 4:(d + 1) * 4],
                              W4[d * P:(d + 1) * P, :])
        wsrcb_sb = big.tile([P, 2 * 4 * P], F16, tag="wsrcb")
        for d in range(2):
            nc.sync.dma_start(wsrcb_sb[:, d * 4 * P:(d + 1) * 4 * P],
                              WSRCB[d * P:(d + 1) * P, :])
        xt_sb = big.tile([P, 2 * n_total], F16, tag="xt")
        nch = 8
        for c in range(nch):
            w = n_total // nch
            for d in range(2):
                nc.sync.dma_start(
                    xt_sb[:, d * n_total + c * w: d * n_total + (c + 1) * w],
                    XT[d * P:(d + 1) * P, c * w:(c + 1) * w])
        at16_sb = big.tile([P, ntiles * rows], F16, tag="at16")
        for jt in range(ntiles):
            nc.sync.dma_start(at16_sb[:, jt * rows:(jt + 1) * rows],
                              AT16[jt * P:(jt + 1) * P, :])

        # ---- SBUF working tiles ----
        negs = big.tile([P, HEADS * rows], F16, tag="negs")     # -s bcast rows
        t3 = big.tile([P, ntiles * 4], F32, tag="t3")           # t per (nt,h)
        s_own = big.tile([P, nib * 4], F32, tag="sown")
        u8 = big.tile([P, nib * 4], F32, tag="u8")              # e^{0.8 s}
        qv = big.tile([P, ntiles * 4], F32, tag="qv")           # e^{0.2 t}
        vv = big.tile([P, ntiles * 4], F32, tag="vv")           # e^{t}
        tbig = big.tile([P, ntiles * 4], F32, tag="tbig")       # 1e4 * t
        fe66 = big.tile([P, ntiles * 4 * 66], F16, tag="fe66")  # feat + ones
        vq16 = big.tile([P, ntiles * VQBLK], F16, tag="vq16")   # panels [q|v]
        vq4 = vq16[:].rearrange("p (n h c) -> p n h c", h=4, c=2 * GRP)
        out_sbs = []
        for ib in range(nib):
            osb = big.tile([P, HEADS * OUT_DIM], F32, tag=f"osb{ib}")
            out_sbs.append(osb)

        # ---- Phase 1a: NEGS + s_own (small matmuls on XTOWN) ----
        with tc.tile_pool(name="pneg", bufs=2, space=bass.MemorySpace.PSUM) as pn, \
             tc.tile_pool(name="pso", bufs=1, space=bass.MemorySpace.PSUM) as pso:
            for h in range(HEADS):
                ps = pn.tile([P, rows], F32, tag="ps_neg")
                for d in range(2):
                    nc.tensor.matmul(
                        ps[:],
                        wsrcb_sb[:, d * 4 * P + h * P: d * 4 * P + (h + 1) * P],
                        xtown_sb[:, d * rows:(d + 1) * rows],
                        start=(d == 0), stop=(d == 1))
                nc.scalar.activation(negs[:, h * rows:(h + 1) * rows],
                                     ps[:], AF.Copy)
            ps_s = pso.tile([P, nib * 4], F32, tag="ps_s")
            for ib in range(nib):
                for d in range(2):
                    nc.tensor.matmul(
                        ps_s[:, ib * 4:(ib + 1) * 4],
                        xtown_sb[:, d * rows + ib * P: d * rows + (ib + 1) * P],
                        w4_sb[:, d * 4:(d + 1) * 4],
                        start=(d == 0), stop=(d == 1))
            nc.vector.tensor_copy(s_own[:], ps_s[:])
        nc.scalar.activation(u8[:], s_own[:], AF.Exp, scale=0.8)

        # panel pad pre-zero (pads 65:68 of each group feed matmuls)
        nc.vector.memset(vq4[:, :, :, 0:2 * GRP].rearrange(
            "p n h (f c) -> p n h f c", c=GRP)[:, :, :, :, 65:68], 0.0)
        # fe66 ones column (65th col = 1.0 so z-mult emits the den column)
        fe66v = fe66[:].rearrange("p (n h c) -> p n h c", h=4, c=66)
        nc.vector.memset(fe66v[:, :, :, 64:65], 1.0)

        # masks: built interleaved with phase 1b (need only negs/t3/at);
        # all 64 tiles stay resident so the matmul loop never waits long.
        m2_pool = ctx.enter_context(tc.tile_pool(name="m2", bufs=8))
        mb_pool = ctx.enter_context(tc.tile_pool(name="mb", bufs=8))
        at16_3 = at16_sb[:].rearrange("p (j i) -> p j i", i=rows)
        SIG_HEADS = ()     # heads whose masks go ACT-sigmoid -> Pool-mult
        m2_tiles = {}

        def make_masks(q):
            tiles = []
            for h in range(HEADS):
                m2t = m2_pool.tile([P, 2 * rows], F16, tag="m2t")
                mbt = mb_pool.tile([P, 2 * rows], F16, tag="mbt")
                for half in range(2):
                    jt = 2 * q + half
                    mbh = mbt[:, half * rows:(half + 1) * rows]
                    if h >= 2 or (h == 1 and half == 0):
                        # step mask via saturated sigmoid on the ACT engine
                        nc.scalar.activation(
                            mbh, negs[:, h * rows:(h + 1) * rows],
                            AF.Sigmoid, scale=-1.0e4,
                            bias=tbig[:, jt * 4 + h: jt * 4 + h + 1])
                    else:
                        nc.vector.tensor_scalar(
                            mbh, negs[:, h * rows:(h + 1) * rows],
                            t3[:, jt * 4 + h: jt * 4 + h + 1],
                            None, AL.is_le)
                # pair-wide multiply by A (one 1024-col 2x op)
                nc.vector.tensor_tensor(
                    m2t[:], mbt[:],
                    at16_sb[:, (2 * q) * rows:(2 * q + 2) * rows], AL.mult)
                tiles.append(m2t)
            m2_tiles[q] = tiles

        # ---- Phase 1b: feat matmuls -> fe66/t3; exps; fp8 panels; masks ----
        CH = 4  # nt per exp chunk
        with tc.tile_pool(name="pfeat", bufs=6, space=bass.MemorySpace.PSUM) as pf:
            for nt0 in range(0, ntiles, CH):
                for nt in range(nt0, nt0 + CH):
                    ps = pf.tile([P, 260], F32, tag="ps_f")
                    for d in range(2):
                        nc.tensor.matmul(
                            ps[:, 0:260],
                            xt_sb[:, d * n_total + nt * P: d * n_total + (nt + 1) * P],
                            w8_sb[:, d * 260:(d + 1) * 260],
                            start=(d == 0), stop=(d == 1))
                    nc.scalar.activation(t3[:, nt * 4:(nt + 1) * 4],
                                         ps[:, 256:260], AF.Copy)
                    nc.scalar.activation(
                        fe66v[:, nt, :, 0:64],
                        ps[:, 0:256].rearrange("p (h c) -> p h c", c=64),
                        AF.Copy)
                ch = slice(nt0 * 4, (nt0 + CH) * 4)
                nc.scalar.activation(qv[:, ch], t3[:, ch], AF.Exp, scale=0.2)
                nc.scalar.activation(vv[:, ch], t3[:, ch], AF.Exp)
                # panels for these source tiles: fp8 q-block (A-branch,
                # wide broadcast on DVE) + f16 [q|v] groups (M-branch)
                for nt in range(nt0, nt0 + CH):
                    for h in range(HEADS):
                        feh = fe66v[:, nt, h, 0:65]
                        nc.vector.tensor_scalar_mul(
                            vq4[:, nt, h, 0:65], feh,
                            qv[:, nt * 4 + h: nt * 4 + h + 1])
                        if (nt + h) % 4 == 0:
                            nc.scalar.activation(
                                vq4[:, nt, h, GRP:GRP + 65], feh, AF.Copy,
                                scale=vv[:, nt * 4 + h: nt * 4 + h + 1])
                        else:
                            nc.vector.tensor_scalar_mul(
                                vq4[:, nt, h, GRP:GRP + 65], feh,
                                vv[:, nt * 4 + h: nt * 4 + h + 1])

        # ---- Phase 2: masked matmuls over j-pairs ----
        nc.vector.tensor_scalar_mul(tbig[:], t3[:], 1.0e4)
        with tc.tile_pool(name="pacc", bufs=1, space=bass.MemorySpace.PSUM) as pa:
            # per ib: tile1 = [A(272) | Mh0(136)], tile2 = [Mh1|Mh2|Mh3]
            t1 = []
            t2 = []
            for ib in range(nib):
                t1_ib = pa.tile([P, 408], F32, tag=f"t1_{ib}")
                t1.append(t1_ib)
            for ib in range(nib):
                t2_ib = pa.tile([P, 408], F32, tag=f"t2_{ib}")
                t2.append(t2_ib)
            # Pre-zero accumulators; all matmuls run start=False because a
            # start=True re-marks the whole tile pending-zero and wipes the
            # other chains' first-pair contributions (3 chains share a tile).
            zsrc = big.tile([P, 408], F32, tag="zsrc")
            nc.vector.memset(zsrc[:], 0.0)
            for ib in range(nib):
                nc.vector.memset(t1[ib][:], 0.0)
                nc.scalar.activation(t2[ib][:], zsrc[:], AF.Copy)

            make_masks(0)
            for q in range(npairs):
                m2_cur = m2_tiles[q]
                if q + 1 < npairs:
                    make_masks(q + 1)
                st = False
                sp = (q == npairs - 1)
                pr = slice(2 * q, 2 * q + 2)
                for ib in range(nib):
                    for half in range(2):
                        jt = 2 * q + half
                        lhsA = at16_3[:, jt, ib * P:(ib + 1) * P]
                        nc.tensor.matmul(t1[ib][:, 0:272], lhsA,
                                         vq4[:, jt, :, 0:GRP],
                                         start=st, stop=sp,
                                         skip_group_check=True)
                    for h in range(HEADS):
                        dst = (t1[ib][:, 272:408] if h == 0
                               else t2[ib][:, (h - 1) * 136: h * 136])
                        for half in range(2):
                            jt = 2 * q + half
                            m2ap = m2_cur[h][
                                :, half * rows + ib * P:
                                half * rows + (ib + 1) * P]
                            rvq = vq4[:, jt, h, 0:136]
                            nc.tensor.matmul(dst, m2ap, rvq,
                                             start=st, stop=sp,
                                             skip_group_check=True)

            # ---- Epilogue: PSUM->SBUF on ACT, combine from SBUF on DVE/Pool ----
            with tc.tile_pool(name="epi", bufs=4) as ep:
                for ib in range(nib):
                    s1 = ep.tile([P, 408], F32, tag="s1")
                    s2 = ep.tile([P, 408], F32, tag="s2")
                    nc.scalar.activation(s1[:], t1[ib][:], AF.Copy)
                    nc.scalar.activation(s2[:], t2[ib][:], AF.Copy)
                    for h in range(HEADS):
                        pa_h = s1[:, h * GRP: h * GRP + 65]
                        mh = (s1[:, 272:408] if h == 0
                              else s2[:, (h - 1) * 136: h * 136])
                        pmq = mh[:, 0:65]
                        pmv = mh[:, GRP:GRP + 65]
                        tmp = ep.tile([P, 65], F32, tag="tmp")
                        nc.vector.scalar_tensor_tensor(
                            tmp[:], pmv, u8[:, ib * 4 + h: ib * 4 + h + 1],
                            pmq, AL.mult, AL.subtract)
                        numer = ep.tile([P, 65], F32, tag="num")
                        nc.gpsimd.tensor_tensor(numer[:], tmp[:], pa_h, AL.add)
                        rc = ep.tile([P, 1], F32, tag="rc")
                        nc.vector.reciprocal(rc[:], numer[:, 64:65])
                        nc.vector.tensor_scalar_mul(
                            out_sbs[ib][:, h * OUT_DIM:(h + 1) * OUT_DIM],
                            numer[:, 0:OUT_DIM], rc[:])
        for ib in range(nib):
            nc.sync.dma_start(OUT[ib * P:(ib + 1) * P, :], out_sbs[ib][:])

    nc.compile()
    return nc


def prep_inputs(X, A, W, a, n_total=N_TOTAL, rows=ROWS, n_cores=N_CORES):
    f16 = np.float16
    X = np.asarray(X, np.float32)
    A = np.asarray(A)
    W = np.asarray(W, np.float32)
    a = np.asarray(a, np.float32)

    XT = np.ascontiguousarray(X.T).astype(f16)
    Wcat = np.ascontiguousarray(W.transpose(1, 0, 2).reshape(IN_DIM, HEADS * OUT_DIM))
    a_src, a_dst = a[:, :OUT_DIM], a[:, OUT_DIM:]
    w_src = np.einsum('hdo,ho->hd', W, a_src).astype(np.float32)
    w_dst = np.einsum('hdo,ho->hd', W, a_dst).astype(np.float32)
    W8 = np.concatenate([Wcat, w_dst.T], axis=1).astype(f16)
    W4 = np.ascontiguousarray(w_src.T).astype(f16)
    WSRCB = np.repeat(-w_src.T[:, :, None], P, axis=2).reshape(IN_DIM, HEADS * P)
    WSRCB = np.ascontiguousarray(WSRCB).astype(f16)

    Af16 = (A > 0).astype(f16)
    in_maps = []
    for c in range(n_cores):
        i0 = c * rows
        at16 = np.ascontiguousarray(Af16[i0:i0 + rows, :].T)
        xtown = np.ascontiguousarray(X[i0:i0 + rows, :].T).astype(f16)
        in_maps.append({
            "XT": XT, "XTOWN": xtown, "W8": W8, "W4": W4,
            "WSRCB": WSRCB, "AT16": at16,
        })
    return in_maps


_CACHED_NC = None


def _get_nc():
    global _CACHED_NC
    if _CACHED_NC is None:
        _CACHED_NC = build_program()
    return _CACHED_NC


def kernel(X, A, W, a, b, _trace=False, _trace_kwargs=None):
    nc = _get_nc()
    in_maps = prep_inputs(X, A, W, a)
    kw = {}
    if _trace:
        kw["trace"] = True
        if _trace_kwargs:
            kw.update(_trace_kwargs)
    res = run_bass_kernel_spmd(nc, in_maps, core_ids=list(range(N_CORES)), **kw)
    out = np.concatenate([r["OUT"] for r in res.results], axis=0)
    out = out + np.asarray(b, np.float32).reshape(1, HEADS * OUT_DIM)
    if _trace:
        return out.astype(np.float32), res
    return out.astype(np.float32)
